# revision 35
# baseline (speedup 1.0000x reference)
"""Trainium2 Bass kernel for nn_Attention_11458972746115.

Multi-head attention (B=1, S=2048, D=1024, H=16, DH=64) with RoPE and a
block-diagonal segment mask, tensor-parallel over heads across 8 NeuronCores
(2 heads per core).  Each core computes qkv projections, RoPE, block-sparse
masked attention and its slice of the output projection; the partial output
products (sum-sharded over the wo contraction) are reduced on the host.

v4 design (vs v3):
 - Generator-based emission scheduler: front (QKV+RoPE), attention and
   output-projection work units are interleaved at instruction granularity so
   the in-order PE queue always has ready matmuls while DVE/Act chew on
   elementwise chains.
 - Paired-head exp: both heads' scores for a window live in one 2-bank psum
   tile ([128, 1024]); a single Act exp covers both.
 - De-aliased PSUM pools: qkv(2) + rope/transpose(1) + scores(2) + pot(2) +
   tail(1) = 8 banks, so phases no longer serialize on shared banks.
 - Renorm once per chunk: r-rows for both heads -> one select-matrix PE
   broadcast -> one reciprocal -> two psum*sbuf muls.
 - RoPE cos/sin muls offloaded to the (otherwise idle) GPSIMD/Pool engine via
   a bf16 drain of the projection psum; the last chunk keeps them on DVE to
   shorten the end-of-kernel dependency chain.
 - Fine-grained startup DMA (per-j x slices) so the first projection matmul
   starts ~0.5us in.
"""

import os
import numpy as np

S = 2048
D = 1024
H = 16
DH = 64
NCORES = 8

_PROG_CACHE = {}


def _chunks(lo, hi, maxw=512):
    # greedy: full-width chunks first so the paired exp has no hole; the
    # remainder chunk (if any) uses two narrow unpaired exps instead.
    out = []
    p = lo
    while p < hi:
        w = min(maxw, hi - p)
        out.append((p, p + w))
        p += w
    return out


def _build(bounds, reps=1):
    import contextlib

    import concourse.bacc as bacc
    import concourse.mybir as mybir
    import concourse.tile as tile
    from concourse.bass import ts

    f32 = mybir.dt.float32
    f32r = mybir.dt.float32r
    bf16 = mybir.dt.bfloat16
    AF = mybir.ActivationFunctionType

    segs = [(bounds[g], bounds[g + 1]) for g in range(4) if bounds[g + 1] > bounds[g]]

    allwins = []
    winidx = {}
    for (lo, hi) in segs:
        lst = []
        for w0 in range(lo, hi, 128):
            w1 = min(w0 + 128, hi)
            lst.append((len(allwins), w0, w1))
            allwins.append((w0, w1))
        winidx[(lo, hi)] = lst
    NW = len(allwins)

    nc = bacc.Bacc(None, target_bir_lowering=False)

    xq_d = nc.dram_tensor("xq3", [128, 8, S], bf16, kind="ExternalInput")
    wq_d = nc.dram_tensor("wq3", [128, 8, 128], bf16, kind="ExternalInput")
    wk_d = nc.dram_tensor("wk3", [128, 8, 128], bf16, kind="ExternalInput")
    wv_d = nc.dram_tensor("wv3", [128, 8, 128], bf16, kind="ExternalInput")
    wo_d = nc.dram_tensor("wo3", [128, 1024], bf16, kind="ExternalInput")
    cs_d = nc.dram_tensor("cs2", [128, S], bf16, kind="ExternalInput")
    sn_d = nc.dram_tensor("sn2", [128, S], bf16, kind="ExternalInput")
    ax_d = nc.dram_tensor("aux2", [128, 256], bf16, kind="ExternalInput")
    out_d = nc.dram_tensor("outp", [S, D], bf16, kind="ExternalOutput")

    with tile.TileContext(nc) as tc:
        rep_ctx = (tc.For_i(0, reps, 1, hint_engines=(
            mybir.EngineType.PE, mybir.EngineType.DVE,
            mybir.EngineType.Activation, mybir.EngineType.SP,
            mybir.EngineType.Pool))
                   if reps > 1 else contextlib.nullcontext())
        with rep_ctx, \
             tc.tile_pool(name="fpq", bufs=1, space="PSUM") as FPQ, \
             tc.tile_pool(name="rp", bufs=1, space="PSUM") as RP, \
             tc.tile_pool(name="stl", bufs=2, space="PSUM") as STL, \
             tc.tile_pool(name="pot", bufs=1, space="PSUM") as POT, \
             tc.tile_pool(name="tlp", bufs=1, space="PSUM") as TLP, \
             tc.tile_pool(name="const", bufs=1) as const, \
             tc.tile_pool(name="pers", bufs=1) as pers, \
             tc.tile_pool(name="qkd", bufs=2) as qkd, \
             tc.tile_pool(name="tpd", bufs=2) as tpdp, \
             tc.tile_pool(name="abp", bufs=2) as abp, \
             tc.tile_pool(name="up", bufs=7) as upool, \
             tc.tile_pool(name="rvp", bufs=2) as rvp, \
             tc.tile_pool(name="ocp", bufs=3) as ocp:

            wq_sb = const.tile([128, 8, 128], bf16)
            wk_sb = const.tile([128, 8, 128], bf16)
            wv_sb = const.tile([128, 8, 128], bf16)
            wo_sb = const.tile([128, 1024], bf16)
            cs_sb = const.tile([128, S], bf16)
            sn_sb = const.tile([128, S], bf16)
            ax_sb = const.tile([128, 256], bf16)
            sel_sb = const.tile([65, 128], bf16)

            xq_sb = pers.tile([128, 8, S], bf16)
            qa = pers.tile([128, S], bf16, name="qa")
            ka = pers.tile([128, S], bf16, name="ka")
            vt_sb = pers.tile([128, S], bf16, name="vt")
            vgc = pers.tile([128, NW, 193], bf16, name="vgc")
            rsb = pers.tile([65, S], bf16, name="rsb")
            osb = pers.tile([128, S], bf16, name="osb")

            # ---- input DMAs, finest first so PE starts early -------------
            nc.sync.dma_start(wq_sb[:, 0:1, :], wq_d[:, 0:1, :])
            nc.sync.dma_start(xq_sb[:, 0, ts(0, 512)], xq_d[:, 0, ts(0, 512)])
            nc.sync.dma_start(wq_sb[:, 1:8, :], wq_d[:, 1:8, :])
            nc.sync.dma_start(xq_sb[:, 1:4, ts(0, 512)], xq_d[:, 1:4, ts(0, 512)])
            nc.sync.dma_start(xq_sb[:, 4:8, ts(0, 512)], xq_d[:, 4:8, ts(0, 512)])
            nc.sync.dma_start(wk_sb[:], wk_d[:])
            nc.sync.dma_start(cs_sb[:, ts(0, 512)], cs_d[:, ts(0, 512)])
            nc.sync.dma_start(sn_sb[:, ts(0, 512)], sn_d[:, ts(0, 512)])
            nc.sync.dma_start(wv_sb[:], wv_d[:])
            nc.sync.dma_start(ax_sb[:], ax_d[:])
            nc.sync.dma_start(xq_sb[:, 0:4, ts(1, 512)], xq_d[:, 0:4, ts(1, 512)])
            nc.sync.dma_start(xq_sb[:, 4:8, ts(1, 512)], xq_d[:, 4:8, ts(1, 512)])
            nc.sync.dma_start(cs_sb[:, 512:S], cs_d[:, 512:S])
            nc.sync.dma_start(sn_sb[:, 512:S], sn_d[:, 512:S])
            nc.sync.dma_start(xq_sb[:, 0:4, ts(2, 512)], xq_d[:, 0:4, ts(2, 512)])
            nc.sync.dma_start(xq_sb[:, 4:8, ts(2, 512)], xq_d[:, 4:8, ts(2, 512)])
            nc.sync.dma_start(wo_sb[:], wo_d[:])
            nc.sync.dma_start(xq_sb[:, 0:4, ts(3, 512)], xq_d[:, 0:4, ts(3, 512)])
            nc.sync.dma_start(xq_sb[:, 4:8, ts(3, 512)], xq_d[:, 4:8, ts(3, 512)])

            P_sb = ax_sb[:, 0:128]
            id_sb = ax_sb[:, 128:256]

            # ---- constant presets ---------------------------------------
            one16 = 0x3F80
            one32 = 0x3F800000
            # vgc: col 64 = h0 ones, col 65 = h1 ones, cols 66:129 zeros
            nc.gpsimd.memset(vgc[:, :, 64:66].bitcast(mybir.dt.uint16), one16)
            nc.gpsimd.memset(vgc[:, :, 66:129].bitcast(mybir.dt.uint16), 0)
            # rsb rows 0:64 = 1.0 so reciprocal of unwritten rows is finite
            nc.gpsimd.memset(rsb[0:64, :].bitcast(mybir.dt.uint16), one16)
            # sel: row 64 -> cols 0:64 (head0 r), row 0 -> cols 64:128 (head1 r)
            nc.gpsimd.memset(sel_sb[:].bitcast(mybir.dt.uint16), 0)
            nc.gpsimd.memset(sel_sb[64:65, 0:64].bitcast(mybir.dt.uint16), one16)
            nc.gpsimd.memset(sel_sb[0:1, 64:128].bitcast(mybir.dt.uint16), one16)

            ntag = [0]

            # ---------------- front: QKV + RoPE + v transposes ------------
            def front_gen(c, fast_rope):
                sl = ts(c, 512)
                qb = FPQ.tile([128, 512], f32, tag=f"qkv{ntag[0] % 2}", name=f"qb{c}")
                ntag[0] += 1
                kb = FPQ.tile([128, 512], f32, tag=f"qkv{ntag[0] % 2}", name=f"kb{c}")
                ntag[0] += 1
                for j in range(8):
                    nc.tensor.matmul(qb[:], wq_sb[:, j, :], xq_sb[:, j, sl],
                                     start=(j == 0), stop=(j == 7))
                    yield
                # q rope elementwise
                qsb = qkd.tile([128, 512], bf16, tag="qs", name=f"qs{c}")
                nc.vector.tensor_copy(qsb[:], qb[:])
                aq = abp.tile([128, 512], bf16, tag="aq", name=f"aq{c}")
                bq = abp.tile([128, 512], bf16, tag="bq", name=f"bq{c}")
                if fast_rope:
                    nc.vector.tensor_mul(aq[:], qsb[:], cs_sb[:, sl])
                    nc.vector.tensor_mul(bq[:], qsb[:], sn_sb[:, sl])
                else:
                    nc.gpsimd.tensor_mul(aq[:], qsb[:], cs_sb[:, sl])
                    nc.gpsimd.tensor_mul(bq[:], qsb[:], sn_sb[:, sl])
                for j in range(8):
                    nc.tensor.matmul(kb[:], wk_sb[:, j, :], xq_sb[:, j, sl],
                                     start=(j == 0), stop=(j == 7))
                    yield
                ksb = qkd.tile([128, 512], bf16, tag="ks", name=f"ks{c}")
                nc.vector.tensor_copy(ksb[:], kb[:])
                ak = abp.tile([128, 512], bf16, tag="ak", name=f"ak{c}")
                bk = abp.tile([128, 512], bf16, tag="bk", name=f"bk{c}")
                if fast_rope:
                    nc.vector.tensor_mul(ak[:], ksb[:], cs_sb[:, sl])
                    nc.vector.tensor_mul(bk[:], ksb[:], sn_sb[:, sl])
                else:
                    nc.gpsimd.tensor_mul(ak[:], ksb[:], cs_sb[:, sl])
                    nc.gpsimd.tensor_mul(bk[:], ksb[:], sn_sb[:, sl])
                vb = FPQ.tile([128, 512], f32, tag=f"qkv{ntag[0] % 2}", name=f"vb{c}")
                ntag[0] += 1
                for j in range(8):
                    nc.tensor.matmul(vb[:], wv_sb[:, j, :], xq_sb[:, j, sl],
                                     start=(j == 0), stop=(j == 7))
                    yield
                rpq = RP.tile([128, 512], f32, tag="rp", name=f"rpq{c}")
                nc.tensor.matmul(rpq[:], P_sb, bq[:], start=True, stop=True)
                yield
                nc.vector.tensor_add(qa[:, sl], rpq[:], aq[:])
                rpk = RP.tile([128, 512], f32, tag="rp", name=f"rpk{c}")
                nc.tensor.matmul(rpk[:], P_sb, bk[:], start=True, stop=True)
                yield
                nc.vector.tensor_add(ka[:, sl], rpk[:], ak[:])
                if c % 2 == 0:
                    nc.scalar.copy(vt_sb[:, sl], vb[:])
                else:
                    nc.vector.tensor_copy(vt_sb[:, sl], vb[:])
                # transposes for windows now fully resident
                loaded = 512 * (c + 1)
                while wdone[0] < NW and allwins[wdone[0]][1] <= loaded:
                    widx = wdone[0]
                    w0, w1 = allwins[widx]
                    wd = w1 - w0
                    wde = min(wd + (wd & 1), S - w0)
                    tp = RP.tile([128, 128], f32, tag="rp", name=f"tp{widx}")
                    nc.tensor.matmul(tp[0:wde, :], vt_sb[:, w0:w0 + wde], id_sb,
                                     start=True, stop=True)
                    yield
                    tpd = tpdp.tile([128, 128], bf16, tag="tpd", name="tpd")
                    if widx % 2 == 0:
                        nc.scalar.copy(tpd[0:wde, :], tp[0:wde, :])
                    else:
                        nc.vector.tensor_copy(tpd[0:wde, :], tp[0:wde, :])
                    nc.gpsimd.tensor_copy(vgc[0:wde, widx, 0:64], tpd[0:wde, 0:64])
                    nc.gpsimd.tensor_copy(vgc[0:wde, widx, 129:193],
                                          tpd[0:wde, 64:128])
                    wdone[0] += 1

            wdone = [0]

            # ---------------- attention ----------------------------------
            # chunk width <= 256 so both heads share one psum bank per
            # window (h0 at cols 0:cwp, h1 at 256:256+cwp) and one exp
            # covers both with a negligible hole.
            def att_gen(lo, hi, clo, chi):
                kts = winidx[(lo, hi)]
                cw = chi - clo
                plo, phi = clo, chi
                if cw % 2:
                    if phi < S:
                        phi += 1
                    else:
                        plo -= 1
                cwp = phi - plo
                off = clo - plo

                pot0 = POT.tile([128, 512], f32, tag="pot0", name="pot0")
                pot1 = POT.tile([128, 512], f32, tag="pot1", name="pot1")

                def _av(ti, win, u):
                    widx, w0, w1 = win
                    wd = w1 - w0
                    st = (ti == 0)
                    sp = (ti == len(kts) - 1)
                    nc.tensor.matmul(pot0[0:65, 0:cwp], vgc[0:wd, widx, 0:65],
                                     u[0:wd, 0:cwp], start=st, stop=sp)
                    yield
                    nc.tensor.matmul(pot1[:, 0:cwp], vgc[0:wd, widx, 65:193],
                                     u[0:wd, 512:512 + cwp], start=st, stop=sp)
                    yield

                qneed = -(-phi // 512) - 1
                pend = None
                for ti, win in enumerate(kts):
                    widx, w0, w1 = win
                    wd = w1 - w0
                    # window w needs the front chunk covering its k range
                    # (ka + vgc transpose) and the q range (qa)
                    yield ("need", max(-(-w1 // 512) - 1, qneed))
                    stl0 = STL.tile([128, 512], f32, tag="stl", name="stl0")
                    nc.tensor.matmul(stl0[0:wd, 0:cwp],
                                     ka[0:64, w0:w1], qa[0:64, plo:phi],
                                     start=True, stop=True)
                    yield
                    stl1 = STL.tile([128, 512], f32, tag="stl", name="stl1")
                    nc.tensor.matmul(stl1[0:wd, 0:cwp],
                                     ka[64:128, w0:w1], qa[64:128, plo:phi],
                                     start=True, stop=True)
                    yield
                    if pend is not None:
                        yield from _av(*pend)
                    u = upool.tile([128, 1024], bf16, tag="u", name="u")
                    nc.scalar.activation(u[0:wd, 0:cwp],
                                         stl0[0:wd, 0:cwp],
                                         AF.Exp, scale=0.125)
                    nc.scalar.activation(u[0:wd, 512:512 + cwp],
                                         stl1[0:wd, 0:cwp],
                                         AF.Exp, scale=0.125)
                    pend = (ti, win, u)
                yield from _av(*pend)
                # renorm (both heads at once)
                nc.scalar.copy(rsb[0:1, plo:phi], pot1[0:1, 0:cwp])
                nc.vector.tensor_copy(rsb[64:65, plo:phi], pot0[64:65, 0:cwp])
                rt = STL.tile([128, 512], f32, tag="stl", name="rt")
                nc.tensor.matmul(rt[:, 0:cwp], sel_sb[:], rsb[0:65, plo:phi],
                                 start=True, stop=True)
                yield
                rvb = rvp.tile([128, 512], f32, tag="rv", name="rvb")
                nc.vector.reciprocal_approx_fast(rvb[:, 0:cwp], rt[:, 0:cwp])
                nc.vector.tensor_mul(osb[0:64, clo:chi],
                                     pot0[0:64, off:off + cw],
                                     rvb[0:64, off:off + cw])
                nc.vector.tensor_mul(osb[64:128, clo:chi],
                                     pot1[64:128, off:off + cw],
                                     rvb[64:128, off:off + cw])

            # ---------------- tail: output projection ---------------------
            tcount = [0]

            def tail_gen(i, endgame):
                oc = ocp.tile([128, 1024], bf16, tag="oc", name="oc")
                for jj in (0, 1):
                    if endgame:
                        m = tcount[0] % 3
                        if m == 0:
                            pout = TLP.tile([128, 512], f32, tag="tl", name="pout")
                        elif m == 1:
                            pout = RP.tile([128, 512], f32, tag="rp", name="pout")
                        else:
                            pout = FPQ.tile([128, 512], f32,
                                            tag=f"qkv{tcount[0] % 2}", name="pout")
                    else:
                        pout = TLP.tile([128, 512], f32, tag="tl", name="pout")
                    tcount[0] += 1
                    nc.tensor.matmul(pout[:], osb[:, ts(i, 128)],
                                     wo_sb[:, ts(jj, 512)],
                                     start=True, stop=True)
                    yield
                    # while attention is in flight keep Act free for exps
                    if att_units:
                        act_share = tcount[0] % 3 == 0
                    else:
                        act_share = tcount[0] % 3 != 0
                    if act_share:
                        nc.scalar.copy(oc[:, ts(jj, 512)], pout[:])
                    else:
                        nc.vector.tensor_copy(oc[:, ts(jj, 512)], pout[:])
                nc.sync.dma_start(out_d[ts(i, 128), :], oc[:])

            # ---------------- emission scheduler --------------------------
            # attention units emit per-window as soon as the front chunk
            # covering that window's k/q columns has been emitted (the pot
            # accumulation is incremental across windows); tails activate as
            # the osb q-prefix completes.
            att_units = []      # [gen, chi, need]
            for (lo, hi) in segs:
                for (clo, chi) in _chunks(lo, hi, 512):
                    att_units.append([att_gen(lo, hi, clo, chi), chi, 0])

            front_q = [front_gen(c, fast_rope=(c == 3)) for c in range(4)]
            tail_q = []
            osb_prefix = [0]
            tail_emitted = [0]
            front_idx = [0]

            def activate_tails(endgame=False):
                while (tail_emitted[0] + 1) * 128 <= osb_prefix[0]:
                    i = tail_emitted[0]
                    tail_q.append(tail_gen(i, endgame or i >= 12))
                    tail_emitted[0] += 1

            def step_front():
                while front_idx[0] < len(front_q):
                    try:
                        next(front_q[front_idx[0]])
                        return True
                    except StopIteration:
                        front_idx[0] += 1
                return False

            def step_tail():
                while tail_q:
                    try:
                        next(tail_q[0])
                        return True
                    except StopIteration:
                        tail_q.pop(0)
                return False

            def step_att():
                while att_units:
                    unit = att_units[0]
                    gen, chi, need = unit
                    if need > front_idx[0] - 1:
                        return False  # blocked on un-emitted front chunk
                    try:
                        r = next(gen)
                        while isinstance(r, tuple) and r[0] == "need":
                            if r[1] > front_idx[0] - 1:
                                unit[2] = r[1]
                                return False
                            r = next(gen)
                        return True
                    except StopIteration:
                        att_units.pop(0)
                        osb_prefix[0] = max(osb_prefix[0], chi)
                        activate_tails()
                return False

            rr = 0
            while True:
                did = False
                # pattern: att, filler, filler  (spacing for Act exp chains)
                if rr % 3 == 0:
                    did = step_att() or step_tail() or step_front()
                elif rr % 3 == 1:
                    did = step_tail() or step_front() or step_att()
                else:
                    did = step_front() or step_tail() or step_att()
                rr += 1
                if not did:
                    break
            activate_tails(endgame=True)
            while step_tail():
                pass

    nc.compile()
    return nc


def _host_tensors(x, seg, fc, fs, wq, wk, wv, wo):
    import ml_dtypes

    bf16 = ml_dtypes.bfloat16

    # cos/sin tables: pair-repeated cos, sign-alternating sin, tiled to 128
    # partitions (the two heads handled per core share the pattern).
    c64 = np.repeat(fc.T, 2, axis=0)
    s64 = np.empty((64, S), np.float32)
    s64[0::2] = fs.T
    s64[1::2] = -fs.T
    cos2 = np.tile(c64, (2, 1)).astype(bf16)
    sin2 = np.tile(s64, (2, 1)).astype(bf16)

    # aux: pair-swap permutation P and identity (for transposes)
    aux = np.zeros((128, 256), np.float32)
    for j in range(128):
        aux[j ^ 1, j] = 1.0          # P
        aux[j, 128 + j] = 1.0        # I
    aux = np.ascontiguousarray(aux).astype(bf16)

    xq3 = np.ascontiguousarray(
        x.T.reshape(8, 128, S).transpose(1, 0, 2)).astype(bf16)

    def wstack(w):
        out = []
        for m in range(NCORES):
            wl = w[m * 128:(m + 1) * 128, :].T.astype(np.float32)
            out.append(np.ascontiguousarray(
                wl.reshape(8, 128, 128).transpose(1, 0, 2)).astype(bf16))
        return out

    wqs = wstack(wq)
    wks = wstack(wk)
    wvs = wstack(wv)
    wos = [np.ascontiguousarray(wo[:, m * 128:(m + 1) * 128].T).astype(bf16)
           for m in range(NCORES)]

    common = {"xq3": xq3, "cs2": cos2, "sn2": sin2, "aux2": aux}
    in_maps = []
    for m in range(NCORES):
        im = dict(common)
        im["wq3"] = wqs[m]
        im["wk3"] = wks[m]
        im["wv3"] = wvs[m]
        im["wo3"] = wos[m]
        in_maps.append(im)
    return in_maps


def kernel(x, seg_ids, freqs_cos, freqs_sin, wq, wk, wv, wo):
    x = np.asarray(x, np.float32).reshape(S, D)
    seg = np.asarray(seg_ids).astype(np.int64)
    fc = np.asarray(freqs_cos, np.float32)
    fs = np.asarray(freqs_sin, np.float32)
    wq = np.asarray(wq, np.float32)
    wk = np.asarray(wk, np.float32)
    wv = np.asarray(wv, np.float32)
    wo = np.asarray(wo, np.float32)

    bounds = tuple(int(b) for b in np.searchsorted(seg, np.arange(5)))
    if bounds not in _PROG_CACHE:
        _PROG_CACHE[bounds] = _build(bounds)
    nc = _PROG_CACHE[bounds]

    in_maps = _host_tensors(x, seg, fc, fs, wq, wk, wv, wo)

    from concourse.bass_utils import run_bass_kernel_spmd

    trace = bool(os.environ.get("BASS_KERNEL_TRACE"))
    res = run_bass_kernel_spmd(nc, in_maps, core_ids=list(range(NCORES)),
                               trace=trace)
    if trace and res.exec_time_ns is not None:
        print(f"HW exec time: {res.exec_time_ns} ns")

    out = np.sum(np.stack([np.asarray(r["outp"], np.float32)
                           for r in res.results]), axis=0)
    return out.astype(np.float32).reshape(1, S, D)


# revision 36
# speedup vs baseline: 1.0159x; 1.0159x over previous
"""Trainium2 Bass kernel for nn_Attention_11458972746115.

Multi-head attention (B=1, S=2048, D=1024, H=16, DH=64) with RoPE and a
block-diagonal segment mask, tensor-parallel over heads across 8 NeuronCores
(2 heads per core).  Each core computes qkv projections, RoPE, block-sparse
masked attention and its slice of the output projection; the partial output
products (sum-sharded over the wo contraction) are reduced on the host.

v4 design (vs v3):
 - Generator-based emission scheduler: front (QKV+RoPE), attention and
   output-projection work units are interleaved at instruction granularity so
   the in-order PE queue always has ready matmuls while DVE/Act chew on
   elementwise chains.
 - Paired-head exp: both heads' scores for a window live in one 2-bank psum
   tile ([128, 1024]); a single Act exp covers both.
 - De-aliased PSUM pools: qkv(2) + rope/transpose(1) + scores(2) + pot(2) +
   tail(1) = 8 banks, so phases no longer serialize on shared banks.
 - Renorm once per chunk: r-rows for both heads -> one select-matrix PE
   broadcast -> one reciprocal -> two psum*sbuf muls.
 - RoPE cos/sin muls offloaded to the (otherwise idle) GPSIMD/Pool engine via
   a bf16 drain of the projection psum; the last chunk keeps them on DVE to
   shorten the end-of-kernel dependency chain.
 - Fine-grained startup DMA (per-j x slices) so the first projection matmul
   starts ~0.5us in.
"""

import os
import numpy as np

S = 2048
D = 1024
H = 16
DH = 64
NCORES = 8

_PROG_CACHE = {}


def _chunks(lo, hi, maxw=512):
    # greedy: full-width chunks first so the paired exp has no hole; the
    # remainder chunk (if any) uses two narrow unpaired exps instead.
    out = []
    p = lo
    while p < hi:
        w = min(maxw, hi - p)
        out.append((p, p + w))
        p += w
    return out


def _build(bounds, reps=1):
    import contextlib

    import concourse.bacc as bacc
    import concourse.mybir as mybir
    import concourse.tile as tile
    from concourse.bass import ts

    f32 = mybir.dt.float32
    f32r = mybir.dt.float32r
    bf16 = mybir.dt.bfloat16
    AF = mybir.ActivationFunctionType

    segs = [(bounds[g], bounds[g + 1]) for g in range(4) if bounds[g + 1] > bounds[g]]

    allwins = []
    winidx = {}
    for (lo, hi) in segs:
        lst = []
        for w0 in range(lo, hi, 128):
            w1 = min(w0 + 128, hi)
            lst.append((len(allwins), w0, w1))
            allwins.append((w0, w1))
        winidx[(lo, hi)] = lst
    NW = len(allwins)

    nc = bacc.Bacc(None, target_bir_lowering=False)

    xq_d = nc.dram_tensor("xq3", [128, 8, S], bf16, kind="ExternalInput")
    wq_d = nc.dram_tensor("wq3", [128, 8, 128], bf16, kind="ExternalInput")
    wk_d = nc.dram_tensor("wk3", [128, 8, 128], bf16, kind="ExternalInput")
    wv_d = nc.dram_tensor("wv3", [128, 8, 128], bf16, kind="ExternalInput")
    wo_d = nc.dram_tensor("wo3", [128, 1024], bf16, kind="ExternalInput")
    cs_d = nc.dram_tensor("cs2", [128, S], bf16, kind="ExternalInput")
    sn_d = nc.dram_tensor("sn2", [128, S], bf16, kind="ExternalInput")
    ax_d = nc.dram_tensor("aux2", [128, 256], bf16, kind="ExternalInput")
    out_d = nc.dram_tensor("outp", [S, D], bf16, kind="ExternalOutput")

    with tile.TileContext(nc) as tc:
        rep_ctx = (tc.For_i(0, reps, 1, hint_engines=(
            mybir.EngineType.PE, mybir.EngineType.DVE,
            mybir.EngineType.Activation, mybir.EngineType.SP,
            mybir.EngineType.Pool))
                   if reps > 1 else contextlib.nullcontext())
        with rep_ctx, \
             tc.tile_pool(name="fpq", bufs=1, space="PSUM") as FPQ, \
             tc.tile_pool(name="rp", bufs=1, space="PSUM") as RP, \
             tc.tile_pool(name="stl", bufs=2, space="PSUM") as STL, \
             tc.tile_pool(name="pot", bufs=1, space="PSUM") as POT, \
             tc.tile_pool(name="tlp", bufs=1, space="PSUM") as TLP, \
             tc.tile_pool(name="const", bufs=1) as const, \
             tc.tile_pool(name="pers", bufs=1) as pers, \
             tc.tile_pool(name="qkd", bufs=2) as qkd, \
             tc.tile_pool(name="tpd", bufs=2) as tpdp, \
             tc.tile_pool(name="abp", bufs=2) as abp, \
             tc.tile_pool(name="up", bufs=7) as upool, \
             tc.tile_pool(name="rvp", bufs=2) as rvp, \
             tc.tile_pool(name="ocp", bufs=3) as ocp:

            wq_sb = const.tile([128, 8, 128], bf16)
            wk_sb = const.tile([128, 8, 128], bf16)
            wv_sb = const.tile([128, 8, 128], bf16)
            wo_sb = const.tile([128, 1024], bf16)
            cs_sb = const.tile([128, S], bf16)
            sn_sb = const.tile([128, S], bf16)
            ax_sb = const.tile([128, 256], bf16)
            sel_sb = const.tile([65, 128], bf16)

            xq_sb = pers.tile([128, 8, S], bf16)
            qa = pers.tile([128, S], bf16, name="qa")
            ka = pers.tile([128, S], bf16, name="ka")
            vt_sb = pers.tile([128, S], bf16, name="vt")
            vgc = pers.tile([128, NW, 193], bf16, name="vgc")
            rsb = pers.tile([65, S], bf16, name="rsb")
            osb = pers.tile([128, S], bf16, name="osb")

            # ---- input DMAs, finest first so PE starts early -------------
            nc.sync.dma_start(wq_sb[:, 0:1, :], wq_d[:, 0:1, :])
            nc.sync.dma_start(xq_sb[:, 0, ts(0, 512)], xq_d[:, 0, ts(0, 512)])
            nc.sync.dma_start(wq_sb[:, 1:8, :], wq_d[:, 1:8, :])
            nc.sync.dma_start(xq_sb[:, 1:4, ts(0, 512)], xq_d[:, 1:4, ts(0, 512)])
            nc.sync.dma_start(xq_sb[:, 4:8, ts(0, 512)], xq_d[:, 4:8, ts(0, 512)])
            nc.sync.dma_start(wk_sb[:], wk_d[:])
            nc.sync.dma_start(cs_sb[:, ts(0, 512)], cs_d[:, ts(0, 512)])
            nc.sync.dma_start(sn_sb[:, ts(0, 512)], sn_d[:, ts(0, 512)])
            nc.sync.dma_start(wv_sb[:], wv_d[:])
            nc.sync.dma_start(ax_sb[:], ax_d[:])
            nc.sync.dma_start(xq_sb[:, 0:4, ts(1, 512)], xq_d[:, 0:4, ts(1, 512)])
            nc.sync.dma_start(xq_sb[:, 4:8, ts(1, 512)], xq_d[:, 4:8, ts(1, 512)])
            nc.sync.dma_start(cs_sb[:, 512:S], cs_d[:, 512:S])
            nc.sync.dma_start(sn_sb[:, 512:S], sn_d[:, 512:S])
            nc.sync.dma_start(xq_sb[:, 0:4, ts(2, 512)], xq_d[:, 0:4, ts(2, 512)])
            nc.sync.dma_start(xq_sb[:, 4:8, ts(2, 512)], xq_d[:, 4:8, ts(2, 512)])
            nc.sync.dma_start(wo_sb[:], wo_d[:])
            nc.sync.dma_start(xq_sb[:, 0:4, ts(3, 512)], xq_d[:, 0:4, ts(3, 512)])
            nc.sync.dma_start(xq_sb[:, 4:8, ts(3, 512)], xq_d[:, 4:8, ts(3, 512)])

            P_sb = ax_sb[:, 0:128]
            id_sb = ax_sb[:, 128:256]

            # ---- constant presets ---------------------------------------
            one16 = 0x3F80
            one32 = 0x3F800000
            # vgc: col 64 = h0 ones, col 65 = h1 ones, cols 66:129 zeros
            nc.gpsimd.memset(vgc[:, :, 64:66].bitcast(mybir.dt.uint16), one16)
            nc.gpsimd.memset(vgc[:, :, 66:129].bitcast(mybir.dt.uint16), 0)
            # rsb rows 0:64 = 1.0 so reciprocal of unwritten rows is finite
            nc.gpsimd.memset(rsb[0:64, :].bitcast(mybir.dt.uint16), one16)
            # sel: row 64 -> cols 0:64 (head0 r), row 0 -> cols 64:128 (head1 r)
            nc.gpsimd.memset(sel_sb[:].bitcast(mybir.dt.uint16), 0)
            nc.gpsimd.memset(sel_sb[64:65, 0:64].bitcast(mybir.dt.uint16), one16)
            nc.gpsimd.memset(sel_sb[0:1, 64:128].bitcast(mybir.dt.uint16), one16)

            ntag = [0]

            # ---------------- front: QKV + RoPE + v transposes ------------
            def front_gen(c, fast_rope):
                sl = ts(c, 512)
                qb = FPQ.tile([128, 512], f32, tag=f"qkv{ntag[0] % 2}", name=f"qb{c}")
                ntag[0] += 1
                kb = FPQ.tile([128, 512], f32, tag=f"qkv{ntag[0] % 2}", name=f"kb{c}")
                ntag[0] += 1
                for j in range(8):
                    nc.tensor.matmul(qb[:], wq_sb[:, j, :], xq_sb[:, j, sl],
                                     start=(j == 0), stop=(j == 7))
                    yield
                # q rope elementwise
                qsb = qkd.tile([128, 512], bf16, tag="qs", name=f"qs{c}")
                nc.vector.tensor_copy(qsb[:], qb[:])
                aq = abp.tile([128, 512], bf16, tag="aq", name=f"aq{c}")
                bq = abp.tile([128, 512], bf16, tag="bq", name=f"bq{c}")
                if fast_rope:
                    nc.vector.tensor_mul(aq[:], qsb[:], cs_sb[:, sl])
                    nc.vector.tensor_mul(bq[:], qsb[:], sn_sb[:, sl])
                else:
                    nc.gpsimd.tensor_mul(aq[:], qsb[:], cs_sb[:, sl])
                    nc.gpsimd.tensor_mul(bq[:], qsb[:], sn_sb[:, sl])
                for j in range(8):
                    nc.tensor.matmul(kb[:], wk_sb[:, j, :], xq_sb[:, j, sl],
                                     start=(j == 0), stop=(j == 7))
                    yield
                ksb = qkd.tile([128, 512], bf16, tag="ks", name=f"ks{c}")
                nc.vector.tensor_copy(ksb[:], kb[:])
                ak = abp.tile([128, 512], bf16, tag="ak", name=f"ak{c}")
                bk = abp.tile([128, 512], bf16, tag="bk", name=f"bk{c}")
                if fast_rope:
                    nc.vector.tensor_mul(ak[:], ksb[:], cs_sb[:, sl])
                    nc.vector.tensor_mul(bk[:], ksb[:], sn_sb[:, sl])
                else:
                    nc.gpsimd.tensor_mul(ak[:], ksb[:], cs_sb[:, sl])
                    nc.gpsimd.tensor_mul(bk[:], ksb[:], sn_sb[:, sl])
                vb = FPQ.tile([128, 512], f32, tag=f"qkv{ntag[0] % 2}", name=f"vb{c}")
                ntag[0] += 1
                for j in range(8):
                    nc.tensor.matmul(vb[:], wv_sb[:, j, :], xq_sb[:, j, sl],
                                     start=(j == 0), stop=(j == 7))
                    yield
                rpq = RP.tile([128, 512], f32, tag="rp", name=f"rpq{c}")
                nc.tensor.matmul(rpq[:], P_sb, bq[:], start=True, stop=True)
                yield
                nc.vector.tensor_add(qa[:, sl], rpq[:], aq[:])
                rpk = RP.tile([128, 512], f32, tag="rp", name=f"rpk{c}")
                nc.tensor.matmul(rpk[:], P_sb, bk[:], start=True, stop=True)
                yield
                nc.vector.tensor_add(ka[:, sl], rpk[:], ak[:])
                if c % 2 == 0:
                    nc.scalar.copy(vt_sb[:, sl], vb[:])
                else:
                    nc.vector.tensor_copy(vt_sb[:, sl], vb[:])
                # transposes for windows now fully resident
                loaded = 512 * (c + 1)
                while wdone[0] < NW and allwins[wdone[0]][1] <= loaded:
                    widx = wdone[0]
                    w0, w1 = allwins[widx]
                    wd = w1 - w0
                    wde = min(wd + (wd & 1), S - w0)
                    tp = RP.tile([128, 128], f32, tag="rp", name=f"tp{widx}")
                    nc.tensor.matmul(tp[0:wde, :], vt_sb[:, w0:w0 + wde], id_sb,
                                     start=True, stop=True)
                    yield
                    tpd = tpdp.tile([128, 128], bf16, tag="tpd", name="tpd")
                    if widx % 2 == 0:
                        nc.scalar.copy(tpd[0:wde, :], tp[0:wde, :])
                    else:
                        nc.vector.tensor_copy(tpd[0:wde, :], tp[0:wde, :])
                    nc.gpsimd.tensor_copy(vgc[0:wde, widx, 0:64], tpd[0:wde, 0:64])
                    nc.gpsimd.tensor_copy(vgc[0:wde, widx, 129:193],
                                          tpd[0:wde, 64:128])
                    wdone[0] += 1

            wdone = [0]

            # ---------------- attention ----------------------------------
            # chunk width <= 256 so both heads share one psum bank per
            # window (h0 at cols 0:cwp, h1 at 256:256+cwp) and one exp
            # covers both with a negligible hole.
            def att_gen(lo, hi, clo, chi):
                kts = winidx[(lo, hi)]
                cw = chi - clo
                plo, phi = clo, chi
                if cw % 2:
                    if phi < S:
                        phi += 1
                    else:
                        plo -= 1
                cwp = phi - plo
                off = clo - plo

                pot0 = POT.tile([128, 512], f32, tag="pot0", name="pot0")
                pot1 = POT.tile([128, 512], f32, tag="pot1", name="pot1")

                def _av(ti, win, u):
                    widx, w0, w1 = win
                    wd = w1 - w0
                    st = (ti == 0)
                    sp = (ti == len(kts) - 1)
                    nc.tensor.matmul(pot0[0:65, 0:cwp], vgc[0:wd, widx, 0:65],
                                     u[0:wd, 0:cwp], start=st, stop=sp)
                    yield
                    nc.tensor.matmul(pot1[:, 0:cwp], vgc[0:wd, widx, 65:193],
                                     u[0:wd, 512:512 + cwp], start=st, stop=sp)
                    yield

                qneed = -(-phi // 512) - 1
                pend = None
                for ti, win in enumerate(kts):
                    widx, w0, w1 = win
                    wd = w1 - w0
                    # window w needs the front chunk covering its k range
                    # (ka + vgc transpose) and the q range (qa)
                    yield ("need", max(-(-w1 // 512) - 1, qneed))
                    stl0 = STL.tile([128, 512], f32, tag="stl", name="stl0")
                    nc.tensor.matmul(stl0[0:wd, 0:cwp],
                                     ka[0:64, w0:w1], qa[0:64, plo:phi],
                                     start=True, stop=True)
                    yield
                    stl1 = STL.tile([128, 512], f32, tag="stl", name="stl1")
                    nc.tensor.matmul(stl1[0:wd, 0:cwp],
                                     ka[64:128, w0:w1], qa[64:128, plo:phi],
                                     start=True, stop=True)
                    yield
                    if pend is not None:
                        yield from _av(*pend)
                    u = upool.tile([128, 1024], bf16, tag="u", name="u")
                    nc.scalar.activation(u[0:wd, 0:cwp],
                                         stl0[0:wd, 0:cwp],
                                         AF.Exp, scale=0.125)
                    nc.scalar.activation(u[0:wd, 512:512 + cwp],
                                         stl1[0:wd, 0:cwp],
                                         AF.Exp, scale=0.125)
                    pend = (ti, win, u)
                yield from _av(*pend)
                # renorm (both heads at once)
                nc.scalar.copy(rsb[0:1, plo:phi], pot1[0:1, 0:cwp])
                nc.vector.tensor_copy(rsb[64:65, plo:phi], pot0[64:65, 0:cwp])
                rt = STL.tile([128, 512], f32, tag="stl", name="rt")
                nc.tensor.matmul(rt[:, 0:cwp], sel_sb[:], rsb[0:65, plo:phi],
                                 start=True, stop=True)
                yield
                rvb = rvp.tile([128, 512], f32, tag="rv", name="rvb")
                nc.vector.reciprocal_approx_fast(rvb[:, 0:cwp], rt[:, 0:cwp])
                nc.vector.tensor_mul(osb[0:64, clo:chi],
                                     pot0[0:64, off:off + cw],
                                     rvb[0:64, off:off + cw])
                nc.vector.tensor_mul(osb[64:128, clo:chi],
                                     pot1[64:128, off:off + cw],
                                     rvb[64:128, off:off + cw])

            # ---------------- tail: output projection ---------------------
            tcount = [0]

            def tail_gen(i, endgame):
                oc = ocp.tile([128, 1024], bf16, tag="oc", name="oc")
                for jj in (0, 1):
                    if endgame:
                        m = tcount[0] % 3
                        if m == 0:
                            pout = TLP.tile([128, 512], f32, tag="tl", name="pout")
                        elif m == 1:
                            pout = RP.tile([128, 512], f32, tag="rp", name="pout")
                        else:
                            pout = FPQ.tile([128, 512], f32,
                                            tag=f"qkv{tcount[0] % 2}", name="pout")
                    else:
                        pout = TLP.tile([128, 512], f32, tag="tl", name="pout")
                    tcount[0] += 1
                    nc.tensor.matmul(pout[:], osb[:, ts(i, 128)],
                                     wo_sb[:, ts(jj, 512)],
                                     start=True, stop=True)
                    yield
                    # while attention is in flight keep Act free for exps
                    if att_units:
                        act_share = tcount[0] % 3 == 0
                    else:
                        act_share = tcount[0] % 3 != 0
                    if act_share:
                        nc.scalar.copy(oc[:, ts(jj, 512)], pout[:])
                    else:
                        nc.vector.tensor_copy(oc[:, ts(jj, 512)], pout[:])
                    nc.sync.dma_start(out_d[ts(i, 128), ts(jj, 512)],
                                      oc[:, ts(jj, 512)])

            # ---------------- emission scheduler --------------------------
            # attention units emit per-window as soon as the front chunk
            # covering that window's k/q columns has been emitted (the pot
            # accumulation is incremental across windows); tails activate as
            # the osb q-prefix completes.
            att_units = []      # [gen, chi, need]
            for (lo, hi) in segs:
                for (clo, chi) in _chunks(lo, hi, 512):
                    att_units.append([att_gen(lo, hi, clo, chi), chi, 0])

            front_q = [front_gen(c, fast_rope=(c == 3)) for c in range(4)]
            tail_q = []
            osb_prefix = [0]
            tail_emitted = [0]
            front_idx = [0]

            def activate_tails(endgame=False):
                while (tail_emitted[0] + 1) * 128 <= osb_prefix[0]:
                    i = tail_emitted[0]
                    tail_q.append(tail_gen(i, endgame or i >= 12))
                    tail_emitted[0] += 1

            def step_front():
                while front_idx[0] < len(front_q):
                    try:
                        next(front_q[front_idx[0]])
                        return True
                    except StopIteration:
                        front_idx[0] += 1
                return False

            def step_tail():
                while tail_q:
                    try:
                        next(tail_q[0])
                        return True
                    except StopIteration:
                        tail_q.pop(0)
                return False

            def step_att():
                while att_units:
                    unit = att_units[0]
                    gen, chi, need = unit
                    if need > front_idx[0] - 1:
                        return False  # blocked on un-emitted front chunk
                    try:
                        r = next(gen)
                        while isinstance(r, tuple) and r[0] == "need":
                            if r[1] > front_idx[0] - 1:
                                unit[2] = r[1]
                                return False
                            r = next(gen)
                        return True
                    except StopIteration:
                        att_units.pop(0)
                        osb_prefix[0] = max(osb_prefix[0], chi)
                        activate_tails()
                return False

            rr = 0
            while True:
                did = False
                # pattern: att, filler, filler  (spacing for Act exp chains)
                if rr % 3 == 0:
                    did = step_att() or step_tail() or step_front()
                elif rr % 3 == 1:
                    did = step_tail() or step_front() or step_att()
                else:
                    did = step_front() or step_tail() or step_att()
                rr += 1
                if not did:
                    break
            activate_tails(endgame=True)
            while step_tail():
                pass

    nc.compile()
    return nc


def _host_tensors(x, seg, fc, fs, wq, wk, wv, wo):
    import ml_dtypes

    bf16 = ml_dtypes.bfloat16

    # cos/sin tables: pair-repeated cos, sign-alternating sin, tiled to 128
    # partitions (the two heads handled per core share the pattern).
    c64 = np.repeat(fc.T, 2, axis=0)
    s64 = np.empty((64, S), np.float32)
    s64[0::2] = fs.T
    s64[1::2] = -fs.T
    cos2 = np.tile(c64, (2, 1)).astype(bf16)
    sin2 = np.tile(s64, (2, 1)).astype(bf16)

    # aux: pair-swap permutation P and identity (for transposes)
    aux = np.zeros((128, 256), np.float32)
    for j in range(128):
        aux[j ^ 1, j] = 1.0          # P
        aux[j, 128 + j] = 1.0        # I
    aux = np.ascontiguousarray(aux).astype(bf16)

    xq3 = np.ascontiguousarray(
        x.T.reshape(8, 128, S).transpose(1, 0, 2)).astype(bf16)

    def wstack(w):
        out = []
        for m in range(NCORES):
            wl = w[m * 128:(m + 1) * 128, :].T.astype(np.float32)
            out.append(np.ascontiguousarray(
                wl.reshape(8, 128, 128).transpose(1, 0, 2)).astype(bf16))
        return out

    wqs = wstack(wq)
    wks = wstack(wk)
    wvs = wstack(wv)
    wos = [np.ascontiguousarray(wo[:, m * 128:(m + 1) * 128].T).astype(bf16)
           for m in range(NCORES)]

    common = {"xq3": xq3, "cs2": cos2, "sn2": sin2, "aux2": aux}
    in_maps = []
    for m in range(NCORES):
        im = dict(common)
        im["wq3"] = wqs[m]
        im["wk3"] = wks[m]
        im["wv3"] = wvs[m]
        im["wo3"] = wos[m]
        in_maps.append(im)
    return in_maps


def kernel(x, seg_ids, freqs_cos, freqs_sin, wq, wk, wv, wo):
    x = np.asarray(x, np.float32).reshape(S, D)
    seg = np.asarray(seg_ids).astype(np.int64)
    fc = np.asarray(freqs_cos, np.float32)
    fs = np.asarray(freqs_sin, np.float32)
    wq = np.asarray(wq, np.float32)
    wk = np.asarray(wk, np.float32)
    wv = np.asarray(wv, np.float32)
    wo = np.asarray(wo, np.float32)

    bounds = tuple(int(b) for b in np.searchsorted(seg, np.arange(5)))
    if bounds not in _PROG_CACHE:
        _PROG_CACHE[bounds] = _build(bounds)
    nc = _PROG_CACHE[bounds]

    in_maps = _host_tensors(x, seg, fc, fs, wq, wk, wv, wo)

    from concourse.bass_utils import run_bass_kernel_spmd

    trace = bool(os.environ.get("BASS_KERNEL_TRACE"))
    res = run_bass_kernel_spmd(nc, in_maps, core_ids=list(range(NCORES)),
                               trace=trace)
    if trace and res.exec_time_ns is not None:
        print(f"HW exec time: {res.exec_time_ns} ns")

    out = np.sum(np.stack([np.asarray(r["outp"], np.float32)
                           for r in res.results]), axis=0)
    return out.astype(np.float32).reshape(1, S, D)


# revision 37
# speedup vs baseline: 1.1027x; 1.0855x over previous
"""Trainium2 Bass kernel for nn_Attention_11458972746115.

Multi-head attention (B=1, S=2048, D=1024, H=16, DH=64) with RoPE and a
block-diagonal segment mask, tensor-parallel over heads across 8 NeuronCores
(2 heads per core).  Each core computes qkv projections, RoPE, block-sparse
masked attention and its slice of the output projection; the partial output
products (sum-sharded over the wo contraction) are reduced on the host.

v8 design (vs v3):
 - Generator-based emission scheduler: front (QKV+RoPE), attention and
   output-projection (tail) work units are interleaved at instruction
   granularity so the in-order PE queue always has ready matmuls while
   DVE/Act chew on elementwise chains.  Attention windows emit as soon as
   the front chunk covering their k/q columns has been emitted (the pot
   psum accumulation is incremental across windows).
 - De-aliased PSUM pools: qkv(2) + rope/transpose(1) + scores(2) +
   pot(2) + tail(1) = 8 banks, so phases no longer serialize on shared
   banks; the endgame tail additionally rotates through the freed
   rope/qkv banks.  (HW constraints found along the way: psum matmul
   writes must be bank-aligned, one pending accumulation group per bank,
   GPSIMD and DMA cannot touch PSUM, matmul operands must be SBUF.)
 - Renorm once per chunk: two r-row copies -> one select-matrix PE
   broadcast -> one reciprocal -> two psum*sbuf muls.
 - RoPE cos/sin muls offloaded to the (otherwise idle) GPSIMD/Pool engine
   via a bf16 drain of the projection psum; the last chunk keeps them on
   DVE to shorten the end-of-kernel dependency chain.  Pool also does the
   v-transpose restructuring copies from a single sbuf drain.
 - Tail psum drains ride DVE while attention is in flight (keeping Act
   free for exp) and Act afterwards.
 - Fine-grained startup DMA (wq j0 + x j0 slices first) and split output
   DMAs so the first matmul starts ~1us in and the final flush is short.
"""

import os
import numpy as np

S = 2048
D = 1024
H = 16
DH = 64
NCORES = 8

_PROG_CACHE = {}


def _chunks(lo, hi, maxw=512):
    # greedy: full-width chunks first so the paired exp has no hole; the
    # remainder chunk (if any) uses two narrow unpaired exps instead.
    out = []
    p = lo
    while p < hi:
        w = min(maxw, hi - p)
        out.append((p, p + w))
        p += w
    return out


def _build(bounds, reps=1):
    import contextlib

    import concourse.bacc as bacc
    import concourse.mybir as mybir
    import concourse.tile as tile
    from concourse.bass import ts

    f32 = mybir.dt.float32
    f32r = mybir.dt.float32r
    bf16 = mybir.dt.bfloat16
    AF = mybir.ActivationFunctionType

    segs = [(bounds[g], bounds[g + 1]) for g in range(4) if bounds[g + 1] > bounds[g]]

    allwins = []
    winidx = {}
    for (lo, hi) in segs:
        lst = []
        for w0 in range(lo, hi, 128):
            w1 = min(w0 + 128, hi)
            lst.append((len(allwins), w0, w1))
            allwins.append((w0, w1))
        winidx[(lo, hi)] = lst
    NW = len(allwins)

    nc = bacc.Bacc(None, target_bir_lowering=False)

    xq_d = nc.dram_tensor("xq3", [128, 8, S], bf16, kind="ExternalInput")
    wq_d = nc.dram_tensor("wq3", [128, 8, 128], bf16, kind="ExternalInput")
    wk_d = nc.dram_tensor("wk3", [128, 8, 128], bf16, kind="ExternalInput")
    wv_d = nc.dram_tensor("wv3", [128, 8, 128], bf16, kind="ExternalInput")
    wo_d = nc.dram_tensor("wo3", [128, 1024], bf16, kind="ExternalInput")
    cs_d = nc.dram_tensor("cs2", [128, S], bf16, kind="ExternalInput")
    sn_d = nc.dram_tensor("sn2", [128, S], bf16, kind="ExternalInput")
    ax_d = nc.dram_tensor("aux2", [128, 256], bf16, kind="ExternalInput")
    out_d = nc.dram_tensor("outp", [S, D], bf16, kind="ExternalOutput")

    with tile.TileContext(nc) as tc:
        rep_ctx = (tc.For_i(0, reps, 1, hint_engines=(
            mybir.EngineType.PE, mybir.EngineType.DVE,
            mybir.EngineType.Activation, mybir.EngineType.SP,
            mybir.EngineType.Pool))
                   if reps > 1 else contextlib.nullcontext())
        with rep_ctx, \
             tc.tile_pool(name="fpq", bufs=1, space="PSUM") as FPQ, \
             tc.tile_pool(name="rp", bufs=1, space="PSUM") as RP, \
             tc.tile_pool(name="stl", bufs=2, space="PSUM") as STL, \
             tc.tile_pool(name="pot", bufs=1, space="PSUM") as POT, \
             tc.tile_pool(name="tlp", bufs=1, space="PSUM") as TLP, \
             tc.tile_pool(name="const", bufs=1) as const, \
             tc.tile_pool(name="pers", bufs=1) as pers, \
             tc.tile_pool(name="qkd", bufs=2) as qkd, \
             tc.tile_pool(name="tpd", bufs=2) as tpdp, \
             tc.tile_pool(name="abp", bufs=2) as abp, \
             tc.tile_pool(name="up", bufs=7) as upool, \
             tc.tile_pool(name="rvp", bufs=2) as rvp, \
             tc.tile_pool(name="ocp", bufs=3) as ocp:

            wq_sb = const.tile([128, 8, 128], bf16)
            wk_sb = const.tile([128, 8, 128], bf16)
            wv_sb = const.tile([128, 8, 128], bf16)
            wo_sb = const.tile([128, 1024], bf16)
            cs_sb = const.tile([128, S], bf16)
            sn_sb = const.tile([128, S], bf16)
            ax_sb = const.tile([128, 256], bf16)
            sel_sb = const.tile([65, 128], bf16)

            xq_sb = pers.tile([128, 8, S], bf16)
            qa = pers.tile([128, S], bf16, name="qa")
            ka = pers.tile([128, S], bf16, name="ka")
            vt_sb = pers.tile([128, S], bf16, name="vt")
            vgc = pers.tile([128, NW, 193], bf16, name="vgc")
            rsb = pers.tile([65, S], bf16, name="rsb")
            osb = pers.tile([128, S], bf16, name="osb")

            # ---- input DMAs, finest first so PE starts early -------------
            nc.sync.dma_start(wq_sb[:, 0:1, :], wq_d[:, 0:1, :])
            nc.sync.dma_start(xq_sb[:, 0, ts(0, 512)], xq_d[:, 0, ts(0, 512)])
            nc.sync.dma_start(wq_sb[:, 1:8, :], wq_d[:, 1:8, :])
            nc.sync.dma_start(xq_sb[:, 1:4, ts(0, 512)], xq_d[:, 1:4, ts(0, 512)])
            nc.sync.dma_start(xq_sb[:, 4:8, ts(0, 512)], xq_d[:, 4:8, ts(0, 512)])
            nc.sync.dma_start(wk_sb[:], wk_d[:])
            nc.sync.dma_start(cs_sb[:, ts(0, 512)], cs_d[:, ts(0, 512)])
            nc.sync.dma_start(sn_sb[:, ts(0, 512)], sn_d[:, ts(0, 512)])
            nc.sync.dma_start(wv_sb[:], wv_d[:])
            nc.sync.dma_start(ax_sb[:], ax_d[:])
            nc.sync.dma_start(xq_sb[:, 0:4, ts(1, 512)], xq_d[:, 0:4, ts(1, 512)])
            nc.sync.dma_start(xq_sb[:, 4:8, ts(1, 512)], xq_d[:, 4:8, ts(1, 512)])
            nc.sync.dma_start(cs_sb[:, 512:S], cs_d[:, 512:S])
            nc.sync.dma_start(sn_sb[:, 512:S], sn_d[:, 512:S])
            nc.sync.dma_start(xq_sb[:, 0:4, ts(2, 512)], xq_d[:, 0:4, ts(2, 512)])
            nc.sync.dma_start(xq_sb[:, 4:8, ts(2, 512)], xq_d[:, 4:8, ts(2, 512)])
            nc.sync.dma_start(wo_sb[:], wo_d[:])
            nc.sync.dma_start(xq_sb[:, 0:4, ts(3, 512)], xq_d[:, 0:4, ts(3, 512)])
            nc.sync.dma_start(xq_sb[:, 4:8, ts(3, 512)], xq_d[:, 4:8, ts(3, 512)])

            P_sb = ax_sb[:, 0:128]
            id_sb = ax_sb[:, 128:256]

            # ---- constant presets ---------------------------------------
            one16 = 0x3F80
            one32 = 0x3F800000
            # vgc: col 64 = h0 ones, col 65 = h1 ones, cols 66:129 zeros
            nc.gpsimd.memset(vgc[:, :, 64:66].bitcast(mybir.dt.uint16), one16)
            nc.gpsimd.memset(vgc[:, :, 66:129].bitcast(mybir.dt.uint16), 0)
            # rsb rows 0:64 = 1.0 so reciprocal of unwritten rows is finite
            nc.gpsimd.memset(rsb[0:64, :].bitcast(mybir.dt.uint16), one16)
            # sel: row 64 -> cols 0:64 (head0 r), row 0 -> cols 64:128 (head1 r)
            nc.gpsimd.memset(sel_sb[:].bitcast(mybir.dt.uint16), 0)
            nc.gpsimd.memset(sel_sb[64:65, 0:64].bitcast(mybir.dt.uint16), one16)
            nc.gpsimd.memset(sel_sb[0:1, 64:128].bitcast(mybir.dt.uint16), one16)

            ntag = [0]

            # ---------------- front: QKV + RoPE + v transposes ------------
            def front_gen(c, fast_rope):
                sl = ts(c, 512)
                qb = FPQ.tile([128, 512], f32, tag=f"qkv{ntag[0] % 2}", name=f"qb{c}")
                ntag[0] += 1
                kb = FPQ.tile([128, 512], f32, tag=f"qkv{ntag[0] % 2}", name=f"kb{c}")
                ntag[0] += 1
                for j in range(8):
                    nc.tensor.matmul(qb[:], wq_sb[:, j, :], xq_sb[:, j, sl],
                                     start=(j == 0), stop=(j == 7))
                    yield
                # q rope elementwise
                qsb = qkd.tile([128, 512], bf16, tag="qs", name=f"qs{c}")
                nc.vector.tensor_copy(qsb[:], qb[:])
                aq = abp.tile([128, 512], bf16, tag="aq", name=f"aq{c}")
                bq = abp.tile([128, 512], bf16, tag="bq", name=f"bq{c}")
                if fast_rope:
                    nc.vector.tensor_mul(aq[:], qsb[:], cs_sb[:, sl])
                    nc.vector.tensor_mul(bq[:], qsb[:], sn_sb[:, sl])
                else:
                    nc.gpsimd.tensor_mul(aq[:], qsb[:], cs_sb[:, sl])
                    nc.gpsimd.tensor_mul(bq[:], qsb[:], sn_sb[:, sl])
                for j in range(8):
                    nc.tensor.matmul(kb[:], wk_sb[:, j, :], xq_sb[:, j, sl],
                                     start=(j == 0), stop=(j == 7))
                    yield
                ksb = qkd.tile([128, 512], bf16, tag="ks", name=f"ks{c}")
                nc.vector.tensor_copy(ksb[:], kb[:])
                ak = abp.tile([128, 512], bf16, tag="ak", name=f"ak{c}")
                bk = abp.tile([128, 512], bf16, tag="bk", name=f"bk{c}")
                if fast_rope:
                    nc.vector.tensor_mul(ak[:], ksb[:], cs_sb[:, sl])
                    nc.vector.tensor_mul(bk[:], ksb[:], sn_sb[:, sl])
                else:
                    nc.gpsimd.tensor_mul(ak[:], ksb[:], cs_sb[:, sl])
                    nc.gpsimd.tensor_mul(bk[:], ksb[:], sn_sb[:, sl])
                vb = FPQ.tile([128, 512], f32, tag=f"qkv{ntag[0] % 2}", name=f"vb{c}")
                ntag[0] += 1
                for j in range(8):
                    nc.tensor.matmul(vb[:], wv_sb[:, j, :], xq_sb[:, j, sl],
                                     start=(j == 0), stop=(j == 7))
                    yield
                rpq = RP.tile([128, 512], f32, tag="rp", name=f"rpq{c}")
                nc.tensor.matmul(rpq[:], P_sb, bq[:], start=True, stop=True)
                yield
                nc.vector.tensor_add(qa[:, sl], rpq[:], aq[:])
                rpk = RP.tile([128, 512], f32, tag="rp", name=f"rpk{c}")
                nc.tensor.matmul(rpk[:], P_sb, bk[:], start=True, stop=True)
                yield
                nc.vector.tensor_add(ka[:, sl], rpk[:], ak[:])
                if c % 2 == 0:
                    nc.scalar.copy(vt_sb[:, sl], vb[:])
                else:
                    nc.vector.tensor_copy(vt_sb[:, sl], vb[:])
                # transposes for windows now fully resident
                loaded = 512 * (c + 1)
                while wdone[0] < NW and allwins[wdone[0]][1] <= loaded:
                    widx = wdone[0]
                    w0, w1 = allwins[widx]
                    wd = w1 - w0
                    wde = min(wd + (wd & 1), S - w0)
                    tp = RP.tile([128, 128], f32, tag="rp", name=f"tp{widx}")
                    nc.tensor.matmul(tp[0:wde, :], vt_sb[:, w0:w0 + wde], id_sb,
                                     start=True, stop=True)
                    yield
                    tpd = tpdp.tile([128, 128], bf16, tag="tpd", name="tpd")
                    if widx % 2 == 0:
                        nc.scalar.copy(tpd[0:wde, :], tp[0:wde, :])
                    else:
                        nc.vector.tensor_copy(tpd[0:wde, :], tp[0:wde, :])
                    nc.gpsimd.tensor_copy(vgc[0:wde, widx, 0:64], tpd[0:wde, 0:64])
                    nc.gpsimd.tensor_copy(vgc[0:wde, widx, 129:193],
                                          tpd[0:wde, 64:128])
                    wdone[0] += 1

            wdone = [0]

            # ---------------- attention ----------------------------------
            # chunk width <= 256 so both heads share one psum bank per
            # window (h0 at cols 0:cwp, h1 at 256:256+cwp) and one exp
            # covers both with a negligible hole.
            def att_gen(lo, hi, clo, chi):
                kts = winidx[(lo, hi)]
                cw = chi - clo
                plo, phi = clo, chi
                if cw % 2:
                    if phi < S:
                        phi += 1
                    else:
                        plo -= 1
                cwp = phi - plo
                off = clo - plo

                pot0 = POT.tile([128, 512], f32, tag="pot0", name="pot0")
                pot1 = POT.tile([128, 512], f32, tag="pot1", name="pot1")

                def _av(ti, win, u):
                    widx, w0, w1 = win
                    wd = w1 - w0
                    st = (ti == 0)
                    sp = (ti == len(kts) - 1)
                    nc.tensor.matmul(pot0[0:65, 0:cwp], vgc[0:wd, widx, 0:65],
                                     u[0:wd, 0:cwp], start=st, stop=sp)
                    yield
                    nc.tensor.matmul(pot1[:, 0:cwp], vgc[0:wd, widx, 65:193],
                                     u[0:wd, 512:512 + cwp], start=st, stop=sp)
                    yield

                qneed = -(-phi // 512) - 1
                pend = None
                for ti, win in enumerate(kts):
                    widx, w0, w1 = win
                    wd = w1 - w0
                    # window w needs the front chunk covering its k range
                    # (ka + vgc transpose) and the q range (qa)
                    yield ("need", max(-(-w1 // 512) - 1, qneed))
                    stl0 = STL.tile([128, 512], f32, tag="stl", name="stl0")
                    nc.tensor.matmul(stl0[0:wd, 0:cwp],
                                     ka[0:64, w0:w1], qa[0:64, plo:phi],
                                     start=True, stop=True)
                    yield
                    stl1 = STL.tile([128, 512], f32, tag="stl", name="stl1")
                    nc.tensor.matmul(stl1[0:wd, 0:cwp],
                                     ka[64:128, w0:w1], qa[64:128, plo:phi],
                                     start=True, stop=True)
                    yield
                    if pend is not None:
                        yield from _av(*pend)
                    u = upool.tile([128, 1024], bf16, tag="u", name="u")
                    nc.scalar.activation(u[0:wd, 0:cwp],
                                         stl0[0:wd, 0:cwp],
                                         AF.Exp, scale=0.125)
                    nc.scalar.activation(u[0:wd, 512:512 + cwp],
                                         stl1[0:wd, 0:cwp],
                                         AF.Exp, scale=0.125)
                    pend = (ti, win, u)
                yield from _av(*pend)
                # renorm (both heads at once)
                nc.scalar.copy(rsb[0:1, plo:phi], pot1[0:1, 0:cwp])
                nc.vector.tensor_copy(rsb[64:65, plo:phi], pot0[64:65, 0:cwp])
                rt = STL.tile([128, 512], f32, tag="stl", name="rt")
                nc.tensor.matmul(rt[:, 0:cwp], sel_sb[:], rsb[0:65, plo:phi],
                                 start=True, stop=True)
                yield
                rvb = rvp.tile([128, 512], f32, tag="rv", name="rvb")
                nc.vector.reciprocal_approx_fast(rvb[:, 0:cwp], rt[:, 0:cwp])
                nc.vector.tensor_mul(osb[0:64, clo:chi],
                                     pot0[0:64, off:off + cw],
                                     rvb[0:64, off:off + cw])
                nc.vector.tensor_mul(osb[64:128, clo:chi],
                                     pot1[64:128, off:off + cw],
                                     rvb[64:128, off:off + cw])

            # ---------------- tail: output projection ---------------------
            tcount = [0]

            def tail_gen(i, endgame):
                oc = ocp.tile([128, 1024], bf16, tag="oc", name="oc")
                for jj in (0, 1):
                    if endgame:
                        m = tcount[0] % 3
                        if m == 0:
                            pout = TLP.tile([128, 512], f32, tag="tl", name="pout")
                        elif m == 1:
                            pout = RP.tile([128, 512], f32, tag="rp", name="pout")
                        else:
                            pout = FPQ.tile([128, 512], f32,
                                            tag=f"qkv{tcount[0] % 2}", name="pout")
                    else:
                        pout = TLP.tile([128, 512], f32, tag="tl", name="pout")
                    tcount[0] += 1
                    nc.tensor.matmul(pout[:], osb[:, ts(i, 128)],
                                     wo_sb[:, ts(jj, 512)],
                                     start=True, stop=True)
                    yield
                    # while attention is in flight keep Act free for exps
                    if att_units:
                        act_share = tcount[0] % 3 == 0
                    else:
                        act_share = tcount[0] % 3 != 0
                    if act_share:
                        nc.scalar.copy(oc[:, ts(jj, 512)], pout[:])
                    else:
                        nc.vector.tensor_copy(oc[:, ts(jj, 512)], pout[:])
                    nc.sync.dma_start(out_d[ts(i, 128), ts(jj, 512)],
                                      oc[:, ts(jj, 512)])

            # ---------------- emission scheduler --------------------------
            # attention units emit per-window as soon as the front chunk
            # covering that window's k/q columns has been emitted (the pot
            # accumulation is incremental across windows); tails activate as
            # the osb q-prefix completes.
            att_units = []      # [gen, chi, need]
            for (lo, hi) in segs:
                for (clo, chi) in _chunks(lo, hi, 512):
                    att_units.append([att_gen(lo, hi, clo, chi), chi, 0])

            front_q = [front_gen(c, fast_rope=(c == 3)) for c in range(4)]
            tail_q = []
            osb_prefix = [0]
            tail_emitted = [0]
            front_idx = [0]

            def activate_tails(endgame=False):
                while (tail_emitted[0] + 1) * 128 <= osb_prefix[0]:
                    i = tail_emitted[0]
                    tail_q.append(tail_gen(i, endgame or i >= 12))
                    tail_emitted[0] += 1

            def step_front():
                while front_idx[0] < len(front_q):
                    try:
                        next(front_q[front_idx[0]])
                        return True
                    except StopIteration:
                        front_idx[0] += 1
                return False

            def step_tail():
                while tail_q:
                    try:
                        next(tail_q[0])
                        return True
                    except StopIteration:
                        tail_q.pop(0)
                return False

            def step_att():
                while att_units:
                    unit = att_units[0]
                    gen, chi, need = unit
                    if need > front_idx[0] - 1:
                        return False  # blocked on un-emitted front chunk
                    try:
                        r = next(gen)
                        while isinstance(r, tuple) and r[0] == "need":
                            if r[1] > front_idx[0] - 1:
                                unit[2] = r[1]
                                return False
                            r = next(gen)
                        return True
                    except StopIteration:
                        att_units.pop(0)
                        osb_prefix[0] = max(osb_prefix[0], chi)
                        activate_tails()
                return False

            rr = 0
            while True:
                did = False
                # pattern: att, filler, filler  (spacing for Act exp chains)
                if rr % 3 == 0:
                    did = step_att() or step_tail() or step_front()
                elif rr % 3 == 1:
                    did = step_tail() or step_front() or step_att()
                else:
                    did = step_front() or step_tail() or step_att()
                rr += 1
                if not did:
                    break
            activate_tails(endgame=True)
            while step_tail():
                pass

    nc.compile()
    return nc


def _host_tensors(x, seg, fc, fs, wq, wk, wv, wo):
    import ml_dtypes

    bf16 = ml_dtypes.bfloat16

    # cos/sin tables: pair-repeated cos, sign-alternating sin, tiled to 128
    # partitions (the two heads handled per core share the pattern).
    c64 = np.repeat(fc.T, 2, axis=0)
    s64 = np.empty((64, S), np.float32)
    s64[0::2] = fs.T
    s64[1::2] = -fs.T
    cos2 = np.tile(c64, (2, 1)).astype(bf16)
    sin2 = np.tile(s64, (2, 1)).astype(bf16)

    # aux: pair-swap permutation P and identity (for transposes)
    aux = np.zeros((128, 256), np.float32)
    for j in range(128):
        aux[j ^ 1, j] = 1.0          # P
        aux[j, 128 + j] = 1.0        # I
    aux = np.ascontiguousarray(aux).astype(bf16)

    xq3 = np.ascontiguousarray(
        x.T.reshape(8, 128, S).transpose(1, 0, 2)).astype(bf16)

    def wstack(w):
        out = []
        for m in range(NCORES):
            wl = w[m * 128:(m + 1) * 128, :].T.astype(np.float32)
            out.append(np.ascontiguousarray(
                wl.reshape(8, 128, 128).transpose(1, 0, 2)).astype(bf16))
        return out

    wqs = wstack(wq)
    wks = wstack(wk)
    wvs = wstack(wv)
    wos = [np.ascontiguousarray(wo[:, m * 128:(m + 1) * 128].T).astype(bf16)
           for m in range(NCORES)]

    common = {"xq3": xq3, "cs2": cos2, "sn2": sin2, "aux2": aux}
    in_maps = []
    for m in range(NCORES):
        im = dict(common)
        im["wq3"] = wqs[m]
        im["wk3"] = wks[m]
        im["wv3"] = wvs[m]
        im["wo3"] = wos[m]
        in_maps.append(im)
    return in_maps


def kernel(x, seg_ids, freqs_cos, freqs_sin, wq, wk, wv, wo):
    x = np.asarray(x, np.float32).reshape(S, D)
    seg = np.asarray(seg_ids).astype(np.int64)
    fc = np.asarray(freqs_cos, np.float32)
    fs = np.asarray(freqs_sin, np.float32)
    wq = np.asarray(wq, np.float32)
    wk = np.asarray(wk, np.float32)
    wv = np.asarray(wv, np.float32)
    wo = np.asarray(wo, np.float32)

    bounds = tuple(int(b) for b in np.searchsorted(seg, np.arange(5)))
    if bounds not in _PROG_CACHE:
        _PROG_CACHE[bounds] = _build(bounds)
    nc = _PROG_CACHE[bounds]

    in_maps = _host_tensors(x, seg, fc, fs, wq, wk, wv, wo)

    from concourse.bass_utils import run_bass_kernel_spmd

    trace = bool(os.environ.get("BASS_KERNEL_TRACE"))
    res = run_bass_kernel_spmd(nc, in_maps, core_ids=list(range(NCORES)),
                               trace=trace)
    if trace and res.exec_time_ns is not None:
        print(f"HW exec time: {res.exec_time_ns} ns")

    out = np.sum(np.stack([np.asarray(r["outp"], np.float32)
                           for r in res.results]), axis=0)
    return out.astype(np.float32).reshape(1, S, D)
